# revision 39
# baseline (speedup 1.0000x reference)
"""Trainium2 Bass kernel for nn_Net_89361089561102 (2-layer dense transformer,
NF4-quantized weights, cls head). Tensor-parallel over 8 NeuronCores.

Strategy (v2):
 - Host: unpack NF4 weights -> bf16 partition-major [P, K/P, M], shard TP-style:
   qkv/gate_up by output dim (heads / ff rows), o/down by INPUT dim (own ctx
   rows / own ff rows) so their outputs are partial sums.
 - Full layer (l=0): ln1+qkv+attention as before (ctx stays in SBUF);
   o_partial -> per-batch-chunk ReduceScatter -> residual add into own x rows
   -> AllGather raw x -> consumer-side rmsnorm -> gated MLP; down_partial ->
   ReduceScatter -> residual -> AllGather -> norm (next ln1). The two token
   chunks (= batches) pipeline through compute/RS/AG so collectives overlap
   compute.
 - Slim layer (l=L-1): only the last token of each batch reaches the output
   through q/o/MLP. k/v full. o/gu/down evaluated at last tokens with
   row-oriented [B, H] math + transposed matmuls (weights as the moving
   operand), residuals folded via (partial + x_last/NC) into two tiny fp32
   AllReduces. Final rmsnorm + cls head computed redundantly per core.
"""

import math
from contextlib import ExitStack
from dataclasses import dataclass

import numpy as np
import ml_dtypes

BF16 = ml_dtypes.bfloat16
EPS = 1e-5
BLK = 64
NF4 = np.array([
    -1.0, -0.6961928009986877, -0.5250730514526367, -0.39491748809814453,
    -0.28444138169288635, -0.18477343022823334, -0.09105003625154495, 0.0,
    0.07958029955625534, 0.16093020141124725, 0.24611230194568634,
    0.33791524171829224, 0.44070982933044434, 0.5626170039176941,
    0.7229568362236023, 1.0], dtype=np.float32)


@dataclass(frozen=True)
class Cfg:
    H: int
    NH: int
    HD: int
    FF: int
    B: int
    S: int
    L: int
    NC: int
    CLS: int = 768
    NCLS: int = 2
    P: int = 128

    @property
    def T(self):
        return self.B * self.S

    @property
    def KT(self):
        return self.H // self.P

    @property
    def HPC(self):  # heads per core
        return self.NH // self.NC

    @property
    def DR(self):  # q/k/v rows per core (own ctx rows)
        return self.HPC * self.HD

    @property
    def DRT(self):
        return self.DR // self.P

    @property
    def OR(self):  # own x rows per core
        return self.H // self.NC

    @property
    def OT(self):
        return self.OR // self.P

    @property
    def FPC(self):  # ff rows per core
        return self.FF // self.NC

    @property
    def FT(self):
        return self.FPC // self.P

    @property
    def MT(self):  # H tiles (partial output rows)
        return self.H // self.P

    @property
    def SP(self):  # seq tiles per batch
        return self.S // self.P

    @property
    def TP_(self):  # token tiles total
        return self.T // self.P

    @property
    def CT(self):
        return self.CLS // self.P

    def nchunks(self, M):
        """split M free-dim into <=512 col chunks"""
        n = (M + 511) // 512
        base = M // n
        assert base * n == M
        return [(i * base, base) for i in range(n)]

    def check(self):
        assert self.H % self.P == 0 and self.FF % self.P == 0
        assert self.S % self.P == 0 and self.S <= 512
        assert self.NH % self.NC == 0 and self.H % self.NC == 0
        assert self.FF % self.NC == 0
        assert self.DR % self.P == 0, "own ctx rows must tile"
        assert self.OR % self.P == 0 and self.FPC % self.P == 0
        assert self.HD <= self.P and self.HD % 2 == 0
        assert self.CLS % self.P == 0


FULL_CFG = Cfg(H=3072, NH=32, HD=96, FF=8192, B=2, S=512, L=2, NC=8)


# ----------------------------------------------------------------------------
# host-side prep
# ----------------------------------------------------------------------------

def dequant_np(packed, absmax, out_f, in_f):
    shifts = (np.arange(8, dtype=np.int32) * 4)
    codes = ((packed[:, None] >> shifts) & 0xF).reshape(-1)
    w = (NF4[codes].reshape(-1, BLK) * absmax[:, None].astype(np.float32))
    return w.reshape(out_f, in_f)


def _wpm(w_t, P):
    """[K, M] fp32 -> [P, K//P, M] bf16 contiguous (partition-major)."""
    K, M = w_t.shape
    return np.ascontiguousarray(
        w_t.reshape(K // P, P, M).transpose(1, 0, 2).astype(BF16))


def host_prep(cfg: Cfg, inputs):
    """Full inputs -> list of per-core input maps."""
    c = cfg
    P = c.P
    x = inputs["embed"][inputs["input_ids"]]          # [B, S, H] fp32
    x0f = np.ascontiguousarray(x.reshape(c.T, c.H).T.astype(np.float32))
    x0 = np.ascontiguousarray(x0f.astype(BF16))        # [H, T] bf16

    # rope tables
    inv = 1.0 / (10000.0 ** (np.arange(0, c.HD, 2, dtype=np.float32) / c.HD))
    f = np.outer(np.arange(c.S, dtype=np.float32), inv)
    emb = np.concatenate([f, f], -1)                   # [S, HD]
    sgn = np.concatenate([-np.ones(c.HD // 2, np.float32),
                          np.ones(c.HD // 2, np.float32)])
    cosT = np.cos(emb).T                               # [HD, S]
    sinT = np.sin(emb).T * sgn[:, None]
    cosT2 = np.ascontiguousarray(
        np.tile(cosT[:, c.S - 1:c.S], (1, c.B)).astype(np.float32))
    sinT2 = np.ascontiguousarray(
        np.tile(sinT[:, c.S - 1:c.S], (1, c.B)).astype(np.float32))
    cosT = np.ascontiguousarray(cosT.astype(np.float32))
    sinT = np.ascontiguousarray(sinT.astype(np.float32))

    am = (inputs["attention_mask"] != 0)               # [B, S]
    tk = np.arange(c.S)
    m1 = np.zeros((c.B, c.SP, P, c.S), np.float32)
    for b in range(c.B):
        for t in range(c.SP):
            rows = tk[t * P:(t + 1) * P]
            m1[b, t] = ((rows[:, None] <= tk[None, :]) & am[b, rows][:, None])
    m1 = m1.astype(BF16)
    am2 = np.zeros((c.B, P, c.SP), np.float32)
    for b in range(c.B):
        am2[b] = am[b].reshape(c.SP, P).T
    am2 = am2.astype(BF16)

    # layernorm weights, full column form [2L+1, P, KT]
    lnw = np.zeros((2 * c.L + 1, P, c.KT), np.float32)
    for l in range(c.L):
        lnw[2 * l] = inputs["ln1_w"][l].reshape(c.KT, P).T
        lnw[2 * l + 1] = inputs["ln2_w"][l].reshape(c.KT, P).T
    lnw[2 * c.L] = inputs["final_ln_w"].reshape(c.KT, P).T

    # identities for PE transposes
    id128b = np.eye(P, dtype=BF16)
    id2b = np.eye(c.B, dtype=BF16)
    id2f = np.eye(c.B, dtype=np.float32)

    # cls head
    w1t = _wpm(inputs["w1"].astype(np.float32).T, P)       # [P, KT, CLS]
    b1row = np.ascontiguousarray(
        np.tile(inputs["b1"][None, :], (c.B, 1)).astype(BF16))
    gcol = np.ascontiguousarray(
        inputs["ln_g"].reshape(c.CT, P).T.astype(np.float32))
    bcol = np.ascontiguousarray(
        inputs["ln_b"].reshape(c.CT, P).T.astype(np.float32))
    w2t = _wpm(inputs["w2"].astype(np.float32).T, P)       # [P, CT, NCLS]
    b2row = np.ascontiguousarray(
        np.tile(inputs["b2"][None, :], (c.B, 1)).astype(np.float32))

    shared = dict(x0=x0, cosT=cosT, sinT=sinT, cosT2=cosT2, sinT2=sinT2,
                  m1=m1, am2=am2, lnw=lnw,
                  id128b=id128b, id2b=id2b, id2f=id2f,
                  w1t=w1t, b1row=b1row, gcol=gcol, bcol=bcol, w2t=w2t,
                  b2row=b2row)

    per_layer = []
    for l in range(c.L):
        wqkv = dequant_np(inputs["qkv_packed"][l], inputs["qkv_absmax"][l],
                          3 * c.H, c.H)
        wo = dequant_np(inputs["o_packed"][l], inputs["o_absmax"][l],
                        c.H, c.H)
        wgu = dequant_np(inputs["gu_packed"][l], inputs["gu_absmax"][l],
                         2 * c.FF, c.H)
        wd = dequant_np(inputs["down_packed"][l], inputs["down_absmax"][l],
                        c.H, c.FF)
        per_layer.append((wqkv, wo, wgu, wd))

    in_maps = []
    for core in range(c.NC):
        m = dict(shared)
        m["x0r"] = np.ascontiguousarray(
            x0f[core * c.OR:(core + 1) * c.OR, :])
        for l in range(c.L):
            wqkv, wo, wgu, wd = per_layer[l]
            d0 = core * c.DR
            m[f"wq{l}"] = _wpm(wqkv[d0:d0 + c.DR, :].T, P)
            m[f"wk{l}"] = _wpm(wqkv[c.H + d0:c.H + d0 + c.DR, :].T, P)
            m[f"wv{l}"] = _wpm(wqkv[2 * c.H + d0:2 * c.H + d0 + c.DR, :].T, P)
            # o: input-sharded over own ctx rows -> [P, DRT, H]
            m[f"wo{l}"] = _wpm(
                np.ascontiguousarray(wo[:, d0:d0 + c.DR].T), P)
            g0 = core * c.FPC
            m[f"wg{l}"] = _wpm(wgu[g0:g0 + c.FPC, :].T, P)
            m[f"wu{l}"] = _wpm(wgu[c.FF + g0:c.FF + g0 + c.FPC, :].T, P)
            # down: input-sharded over own ff rows -> [P, FT, H]
            m[f"wd{l}"] = _wpm(
                np.ascontiguousarray(wd[:, g0:g0 + c.FPC].T), P)
        in_maps.append(m)
    return in_maps


# ----------------------------------------------------------------------------
# device kernel
# ----------------------------------------------------------------------------

def build_nc(cfg: Cfg):
    import concourse.bass as bass
    import concourse.mybir as mybir
    import concourse.tile as tile
    from concourse import bacc

    c = cfg
    c.check()
    P = c.P
    f32 = mybir.dt.float32
    bf16 = mybir.dt.bfloat16
    AF = mybir.ActivationFunctionType
    OP = mybir.AluOpType

    nc = bacc.Bacc("TRN2", target_bir_lowering=False, debug=False,
                   enable_asserts=False, num_devices=c.NC)
    RG = [list(range(c.NC))]
    SHARED = "Shared" if c.NC > 4 else "Local"

    def din(name, shape, dt):
        return nc.dram_tensor(name, list(shape), dt, kind="ExternalInput").ap()

    x0 = din("x0", [c.H, c.T], bf16)
    x0r = din("x0r", [c.OR, c.T], f32)
    cosT = din("cosT", [c.HD, c.S], f32)
    sinT = din("sinT", [c.HD, c.S], f32)
    cosT2 = din("cosT2", [c.HD, c.B], f32)
    sinT2 = din("sinT2", [c.HD, c.B], f32)
    m1 = din("m1", [c.B, c.SP, P, c.S], bf16)
    am2 = din("am2", [c.B, P, c.SP], bf16)
    lnw_d = din("lnw", [2 * c.L + 1, P, c.KT], f32)
    id128b_d = din("id128b", [P, P], bf16)
    id2b_d = din("id2b", [c.B, c.B], bf16)
    id2f_d = din("id2f", [c.B, c.B], f32)
    w1t = din("w1t", [P, c.KT, c.CLS], bf16)
    b1row_d = din("b1row", [c.B, c.CLS], bf16)
    gcol_d = din("gcol", [P, c.CT], f32)
    bcol_d = din("bcol", [P, c.CT], f32)
    w2t_d = din("w2t", [P, c.CT, c.NCLS], bf16)
    b2row_d = din("b2row", [c.B, c.NCLS], f32)
    wq = [din(f"wq{l}", [P, c.KT, c.DR], bf16) for l in range(c.L)]
    wk = [din(f"wk{l}", [P, c.KT, c.DR], bf16) for l in range(c.L)]
    wv = [din(f"wv{l}", [P, c.KT, c.DR], bf16) for l in range(c.L)]
    wo = [din(f"wo{l}", [P, c.DRT, c.H], bf16) for l in range(c.L)]
    wg = [din(f"wg{l}", [P, c.KT, c.FPC], bf16) for l in range(c.L)]
    wu = [din(f"wu{l}", [P, c.KT, c.FPC], bf16) for l in range(c.L)]
    wd = [din(f"wd{l}", [P, c.FT, c.H], bf16) for l in range(c.L)]
    out_d = nc.dram_tensor("logits_out", [c.B, c.NCLS], f32,
                           kind="ExternalOutput").ap()

    isqrt_hd = 1.0 / math.sqrt(c.HD)
    HCH = c.nchunks(c.H)          # H free-dim chunks (slim rows)
    FCH = c.nchunks(c.FPC)        # FPC chunks
    CCH = c.nchunks(c.CLS)        # CLS chunks

    with tile.TileContext(nc) as tc, ExitStack() as ctx:
        const = ctx.enter_context(tc.tile_pool(name="const", bufs=1))
        persist = ctx.enter_context(tc.tile_pool(name="persist", bufs=1))
        wpool = ctx.enter_context(tc.tile_pool(name="wpool", bufs=2))
        xpool = ctx.enter_context(tc.tile_pool(name="xpool", bufs=3))
        spool = ctx.enter_context(tc.tile_pool(name="spool", bufs=2))
        ppool = ctx.enter_context(tc.tile_pool(name="ppool", bufs=2))
        rpool = ctx.enter_context(tc.tile_pool(name="rpool", bufs=1))
        psum = ctx.enter_context(tc.tile_pool(name="psum", bufs=8,
                                              space="PSUM"))
        dram = ctx.enter_context(tc.tile_pool(name="dram", bufs=1,
                                              space="DRAM"))

        # ---- constants in SBUF ----
        ones_c32 = const.tile([P, 1], f32, tag="ones_c32")
        nc.vector.memset(ones_c32[:], 1.0)
        ones_cbf = const.tile([P, 1], bf16, tag="ones_cbf")
        nc.vector.memset(ones_cbf[:], 1.0)
        ones_r32 = const.tile([1, P], f32, tag="ones_r32")
        nc.vector.memset(ones_r32[:], 1.0)
        eps_col = const.tile([P, 1], f32, tag="eps_col")
        nc.vector.memset(eps_col[:], EPS)
        invnc_col = const.tile([c.B, 1], f32, tag="invnc_col")
        nc.vector.memset(invnc_col[:], 1.0 / c.NC)
        cos_sb = const.tile([c.HD, c.S], f32, tag="cos_sb")
        nc.sync.dma_start(out=cos_sb[:], in_=cosT)
        sin_sb = const.tile([c.HD, c.S], f32, tag="sin_sb")
        nc.sync.dma_start(out=sin_sb[:], in_=sinT)
        cos2_sb = const.tile([c.HD, c.B], f32, tag="cos2_sb")
        nc.sync.dma_start(out=cos2_sb[:], in_=cosT2)
        sin2_sb = const.tile([c.HD, c.B], f32, tag="sin2_sb")
        nc.sync.dma_start(out=sin2_sb[:], in_=sinT2)
        am2_sb = const.tile([P, c.B, c.SP], bf16, tag="am2_sb")
        for b in range(c.B):
            nc.sync.dma_start(out=am2_sb[:, b, :], in_=am2[b])
        lnw_sb = const.tile([P, 2 * c.L + 1, c.KT], f32, tag="lnw_sb")
        for n in range(2 * c.L + 1):
            nc.sync.dma_start(out=lnw_sb[:, n, :], in_=lnw_d[n])
        id128b_sb = const.tile([P, P], bf16, tag="id128b_sb")
        nc.sync.dma_start(out=id128b_sb[:], in_=id128b_d)
        id2b_sb = const.tile([c.B, c.B], bf16, tag="id2b_sb")
        nc.sync.dma_start(out=id2b_sb[:], in_=id2b_d)
        id2f_sb = const.tile([c.B, c.B], f32, tag="id2f_sb")
        nc.sync.dma_start(out=id2f_sb[:], in_=id2f_d)
        b1row_sb = const.tile([c.B, c.CLS], bf16, tag="b1row_sb")
        nc.sync.dma_start(out=b1row_sb[:], in_=b1row_d)
        gcol_sb = const.tile([P, c.CT], f32, tag="gcol_sb")
        nc.sync.dma_start(out=gcol_sb[:], in_=gcol_d)
        bcol_sb = const.tile([P, c.CT], f32, tag="bcol_sb")
        nc.sync.dma_start(out=bcol_sb[:], in_=bcol_d)
        w2t_sb = const.tile([P, c.CT, c.NCLS], bf16, tag="w2t_sb")
        nc.sync.dma_start(out=w2t_sb[:], in_=w2t_d)
        b2row_sb = const.tile([c.B, c.NCLS], f32, tag="b2row_sb")
        nc.sync.dma_start(out=b2row_sb[:], in_=b2row_d)

        # ---- collective warm-up: absorb channel-establish cost under
        # the first compute phase ----
        wu_sb = const.tile([P, 512], bf16, tag="wu_sb")
        nc.vector.memset(wu_sb[:], 0.0)
        wu_in = dram.tile([P, 512], bf16, tag="wu_in", name="wu_in")
        wu_out = dram.tile([P * c.NC, 512], bf16, addr_space=SHARED,
                           tag="wu_out", name="wu_out")
        nc.sync.dma_start(out=wu_in[:], in_=wu_sb[:])
        nc.gpsimd.collective_compute(
            "AllGather", OP.bypass, replica_groups=RG,
            ins=[wu_in[:]], outs=[wu_out[:]])
        wu2_in = dram.tile([c.NC, 512], bf16, tag="wu2_in", name="wu2_in")
        wu2_out = dram.tile([1, 512], bf16, tag="wu2_out", name="wu2_out")
        nc.sync.dma_start(out=wu2_in[:], in_=wu_sb[0:c.NC, :])
        nc.gpsimd.collective_compute(
            "ReduceScatter", OP.add, replica_groups=RG,
            ins=[wu2_in[:]], outs=[wu2_out[:]])
        wu3_in = dram.tile([1, 512], bf16, tag="wu3_in", name="wu3_in")
        wu3_out = dram.tile([1, 512], bf16, addr_space=SHARED,
                            tag="wu3_out", name="wu3_out")
        nc.sync.dma_start(out=wu3_in[:], in_=wu_sb[0:1, :])
        nc.gpsimd.collective_compute(
            "AllReduce", OP.add, replica_groups=RG,
            ins=[wu3_in[:]], outs=[wu3_out[:]])

        # ---- persistent activation state ----
        xn = persist.tile([P, c.KT, c.T], bf16, tag="xn")  # normalized x
        xrows = persist.tile([P, c.OT, c.T], f32, tag="xrows")  # own raw x
        for ot in range(c.OT):
            nc.sync.dma_start(out=xrows[:, ot, :],
                              in_=x0r[ot * P:(ot + 1) * P, :])

        # ---------- helpers ----------
        def emit_norm(src_ap, lnidx, ncols, col0, chunks, tag,
                      cap_dst=None, cap_col=0):
            """rmsnorm of src [H, ncols] (bf16 dram) -> xn[:, :, col0:+ncols].
            chunks: list of (c0, cw) splitting ncols for psum rows.
            cap_dst: optionally capture raw last column into [P, KT, B]."""
            ss = [psum.tile([1, cw], f32, tag="ps", name=f"ss{tag}{ci}")
                  for ci, (c0, cw) in enumerate(chunks)]
            for kt in range(c.KT):
                xf = xpool.tile([P, ncols], bf16, tag="xf", name=f"xf{tag}",
                                bufs=3)
                nc.sync.dma_start(out=xf[:], in_=src_ap[kt * P:(kt + 1) * P, :])
                nc.vector.tensor_copy(xn[:, kt, col0:col0 + ncols], xf[:])
                if cap_dst is not None:
                    nc.vector.tensor_copy(cap_dst[:, kt, cap_col:cap_col + 1],
                                          xf[:, ncols - 1:ncols])
                sq = xpool.tile([P, ncols], bf16, tag="sq", name=f"sq{tag}",
                                bufs=3)
                nc.vector.tensor_mul(sq[:], xf[:], xf[:])
                for ci, (c0, cw) in enumerate(chunks):
                    nc.tensor.matmul(ss[ci][:], ones_cbf[:], sq[:, c0:c0 + cw],
                                     start=(kt == 0), stop=(kt == c.KT - 1))
            bc = spool.tile([P, ncols], f32, tag="bc", name=f"bc{tag}",
                            bufs=1)
            for ci, (c0, cw) in enumerate(chunks):
                lt = spool.tile([1, cw], f32, tag="lt", name=f"lt{tag}",
                                bufs=1)
                nc.scalar.activation(lt[:], ss[ci][:], AF.Ln,
                                     bias=eps_col[0:1, :], scale=1.0 / c.H)
                rt = spool.tile([1, cw], f32, tag="dr", name=f"rt{tag}",
                                bufs=1)
                nc.scalar.activation(rt[:], lt[:], AF.Exp, scale=-0.5)
                bb = psum.tile([P, cw], f32, tag="ps", name=f"bb{tag}{ci}")
                nc.tensor.matmul(bb[:], ones_r32[:], rt[:],
                                 start=True, stop=True)
                nc.scalar.copy(bc[:, c0:c0 + cw], bb[:])
            for kt in range(c.KT):
                sl = xn[:, kt, col0:col0 + ncols]
                nc.vector.scalar_tensor_tensor(
                    sl, sl, lnw_sb[:, lnidx, kt:kt + 1], bc[:],
                    OP.mult, OP.mult)

        def kouter_pass(KK, wsrc, wcols, groups, rhs_fn, rhs_load=None,
                        name="kp"):
            """Contraction over KK k-tiles, streaming partition-major weights.
            groups: list of (lhs_c0, lhs_cw, out_n, rhs_key)."""
            ps = [psum.tile([cw, n], f32, tag="ps", name=f"{name}{gi}")
                  for gi, (c0, cw, n, rk) in enumerate(groups)]
            G = max(1, min(8, 2048 // wcols))
            for k0 in range(0, KK, G):
                g_n = min(G, KK - k0)
                wt = wpool.tile([P, G * wcols], bf16, tag="wt",
                                name=f"{name}w")
                wt3 = wt[:].rearrange("p (g m) -> p g m", g=G)
                nc.sync.dma_start(out=wt3[:, 0:g_n, :], in_=wsrc(k0, g_n))
                for g in range(g_n):
                    kt = k0 + g
                    rl = rhs_load(kt) if rhs_load is not None else None
                    for gi, (c0, cw, n, rk) in enumerate(groups):
                        nc.tensor.matmul(ps[gi][:], wt3[:, g, c0:c0 + cw],
                                         rhs_fn(kt, rk, rl),
                                         start=(kt == 0), stop=(kt == KK - 1))
            return ps

        def emit_rope(src_ps, qr_dst, cos_ap, sin_ap, ncols):
            """rope: qr_dst = src*cos + swap_half(src)*sin_signed."""
            h2 = c.HD // 2
            qs = rpool.tile([c.HD, ncols], bf16, tag="qs", name="qs")
            nc.vector.tensor_copy(qs[:], src_ps[:])
            rot = rpool.tile([c.HD, ncols], bf16, tag="rot", name="rot")
            nc.sync.dma_start(out=rot[0:h2, :], in_=qs[h2:c.HD, :])
            nc.sync.dma_start(out=rot[h2:c.HD, :], in_=qs[0:h2, :])
            nc.vector.tensor_mul(qs[:], qs[:], cos_ap)
            nc.vector.tensor_mul(rot[:], rot[:], sin_ap)
            nc.vector.tensor_add(qr_dst, qs[:], rot[:])

        def part_store(ps_list, m0, xp_dram, bcols, tag):
            """psum partial tiles (8 m-tiles starting at m0) -> DRAM rows."""
            for mi, pst in enumerate(ps_list):
                st = xpool.tile([P, bcols], bf16, tag="pst", name=f"st{tag}",
                                bufs=2)
                nc.scalar.copy(st[:], pst[:])
                r0 = (m0 + mi) * P
                nc.sync.dma_start(out=xp_dram[r0:r0 + P, :], in_=st[:])

        def rs_add_ag(xp_b, b, tag):
            """RS partial [H,S] -> own rows; add residual into xrows; AG raw
            x rows back to a [H,S] shared buffer. Returns xg dram AP."""
            rsg = dram.tile([c.OR, c.S], bf16,
                            tag=f"rsg{tag}", name=f"rsg{tag}")
            nc.gpsimd.collective_compute(
                "ReduceScatter", OP.add, replica_groups=RG,
                ins=[xp_b[:]], outs=[rsg[:]])
            xrb = dram.tile([c.OR, c.S], bf16, tag=f"xrb{tag}",
                            name=f"xrb{tag}")
            for ot in range(c.OT):
                rl = xpool.tile([P, c.S], bf16, tag="rsl", name=f"rsl{tag}",
                                bufs=2)
                nc.sync.dma_start(out=rl[:],
                                  in_=rsg[ot * P:(ot + 1) * P, :])
                xsl = xrows[:, ot, b * c.S:(b + 1) * c.S]
                nc.vector.tensor_add(xsl, xsl, rl[:])
                st = xpool.tile([P, c.S], bf16, tag="rsl", name=f"rst{tag}",
                                bufs=2)
                nc.vector.tensor_copy(st[:], xsl)
                nc.sync.dma_start(out=xrb[ot * P:(ot + 1) * P, :], in_=st[:])
            xg = dram.tile([c.H, c.S], bf16, addr_space=SHARED,
                           tag=f"xg{tag}", name=f"xg{tag}")
            nc.gpsimd.collective_compute(
                "AllGather", OP.bypass, replica_groups=RG,
                ins=[xrb[:]], outs=[xg[:]])
            return xg

        # ================= layer 0 .. L-2 (full layers) =================
        # initial norm from replicated x0
        full_chunks = [(b * c.S, c.S) for b in range(c.B)]
        emit_norm(x0, 0, c.T, 0, full_chunks, tag="i")

        for l in range(c.L - 1):
            # ---- qkv ----
            q_rot = persist.tile([c.HD, c.HPC, c.T], bf16, tag="qrot",
                                 name=f"qrot{l}")
            k_rot = persist.tile([c.HD, c.HPC, c.T], bf16, tag="krot",
                                 name=f"krot{l}")
            v_sb = persist.tile([P, c.TP_, c.DR], bf16, tag="vsb",
                                name=f"vsb{l}")

            qg = [(h * c.HD, c.HD, c.S, b)
                  for h in range(c.HPC) for b in range(c.B)]
            qrhs = lambda kt, rk, rl: xn[:, kt, rk * c.S:(rk + 1) * c.S]
            qps = kouter_pass(c.KT, lambda k0, n: wq[l][:, k0:k0 + n, :],
                              c.DR, qg, qrhs, name="qp")
            for gi, (c0, cw, n, rk) in enumerate(qg):
                h = c0 // c.HD
                emit_rope(qps[gi], q_rot[:, h, rk * c.S:(rk + 1) * c.S],
                          cos_sb[:], sin_sb[:], c.S)
            kps = kouter_pass(c.KT, lambda k0, n: wk[l][:, k0:k0 + n, :],
                              c.DR, qg, qrhs, name="kp")
            for gi, (c0, cw, n, rk) in enumerate(qg):
                h = c0 // c.HD
                emit_rope(kps[gi], k_rot[:, h, rk * c.S:(rk + 1) * c.S],
                          cos_sb[:], sin_sb[:], c.S)
            vps = [psum.tile([P, c.DR], f32, tag="ps", name=f"vp{tt}")
                   for tt in range(c.TP_)]
            G = max(1, min(8, 2048 // c.DR))
            for k0 in range(0, c.KT, G):
                g_n = min(G, c.KT - k0)
                wt = wpool.tile([P, G * c.DR], bf16, tag="wt", name="vw")
                wt3 = wt[:].rearrange("p (g m) -> p g m", g=G)
                nc.sync.dma_start(out=wt3[:, 0:g_n, :],
                                  in_=wv[l][:, k0:k0 + g_n, :])
                for g in range(g_n):
                    kt = k0 + g
                    for tt in range(c.TP_):
                        nc.tensor.matmul(vps[tt][:],
                                         xn[:, kt, tt * P:(tt + 1) * P],
                                         wt3[:, g, :],
                                         start=(kt == 0), stop=(kt == c.KT - 1))
            for tt in range(c.TP_):
                nc.scalar.copy(v_sb[:, tt, :], vps[tt][:])

            # ---- attention (ctx stays in SBUF, tiled to 128 rows) ----
            ctx_sb = persist.tile([P, c.DRT, c.T], bf16, tag="ctx_sb",
                                  name=f"ctx{l}")
            for b in range(c.B):
                mask_sb = ppool.tile([P, c.SP, c.S], bf16, tag="maskb",
                                     name=f"maskb{l}{b}", bufs=1)
                for t in range(c.SP):
                    nc.sync.dma_start(out=mask_sb[:, t, :], in_=m1[b, t])
                for h in range(c.HPC):
                    den = psum.tile([1, c.S], f32, tag="ps", name="den")
                    cps = psum.tile([c.HD, c.S], f32, tag="ps", name="cps")
                    for t in range(c.SP):
                        sps = psum.tile([P, c.S], f32, tag="ps", name="sps")
                        nc.tensor.matmul(
                            sps[:],
                            k_rot[:, h, b * c.S + t * P:
                                  b * c.S + (t + 1) * P],
                            q_rot[:, h, b * c.S:(b + 1) * c.S],
                            start=True, stop=True)
                        pt = ppool.tile([P, c.S], bf16, tag="pt", name="pt")
                        nc.scalar.activation(pt[:], sps[:], AF.Exp,
                                             scale=isqrt_hd)
                        nc.vector.tensor_mul(
                            pt[:], pt[:], mask_sb[:, t, :])
                        nc.tensor.matmul(den[:], ones_cbf[:], pt[:],
                                         start=(t == 0),
                                         stop=(t == c.SP - 1))
                        nc.tensor.matmul(
                            cps[:],
                            v_sb[:, b * c.SP + t,
                                 h * c.HD:(h + 1) * c.HD],
                            pt[:],
                            start=(t == 0), stop=(t == c.SP - 1))
                    dr = spool.tile([1, c.S], f32, tag="dr", name="dr",
                                    bufs=1)
                    nc.vector.reciprocal(dr[:], den[:])
                    bb = psum.tile([c.HD, c.S], f32, tag="ps", name="bb")
                    nc.tensor.matmul(bb[:], ones_r32[:, 0:c.HD], dr[:],
                                     start=True, stop=True)
                    bsb = spool.tile([c.HD, c.S], bf16, tag="bsb",
                                     name="bsb", bufs=2)
                    nc.vector.tensor_copy(bsb[:], bb[:])
                    csb = spool.tile([c.HD, c.S], bf16, tag="csb",
                                     name="csb", bufs=1)
                    nc.vector.tensor_mul(csb[:], cps[:], bsb[:])
                    # scatter [HD, S] into 128-row ctx tiles (partition DMA)
                    r0 = h * c.HD
                    while r0 < (h + 1) * c.HD:
                        kt = r0 // P
                        pr0 = r0 % P
                        take = min((h + 1) * c.HD - r0, P - pr0)
                        nc.sync.dma_start(
                            out=ctx_sb[pr0:pr0 + take, kt,
                                       b * c.S:(b + 1) * c.S],
                            in_=csb[r0 - h * c.HD:r0 - h * c.HD + take, :])
                        r0 += take

            # ---- o partial (input-sharded) + RS/AG transition ----
            xgs = []
            for b in range(c.B):
                xpo = dram.tile([c.H, c.S], bf16, tag=f"xpo{l}_{b}",
                                name=f"xpo{l}_{b}")
                for m0 in range(0, c.MT, 8):
                    mtn = min(8, c.MT - m0)
                    ops_ = [psum.tile([P, c.S], f32, tag="ps",
                                      name=f"op{b}_{m0}_{mi}")
                            for mi in range(mtn)]
                    for kt in range(c.DRT):
                        wt = wpool.tile([P, mtn * P], bf16, tag="wt",
                                        name=f"ow{b}{m0}")
                        nc.sync.dma_start(
                            out=wt[:],
                            in_=wo[l][:, kt, m0 * P:(m0 + mtn) * P])
                        for mi in range(mtn):
                            nc.tensor.matmul(
                                ops_[mi][:], wt[:, mi * P:(mi + 1) * P],
                                ctx_sb[:, kt, b * c.S:(b + 1) * c.S],
                                start=(kt == 0), stop=(kt == c.DRT - 1))
                    part_store(ops_, m0, xpo, c.S, tag=f"o{l}{b}{m0}")
                xgs.append(rs_add_ag(xpo, b, tag=f"o{l}_{b}"))

            # ---- gated MLP: norm chunk b immediately followed by its
            # consumers (g/u/down) so chunk 1's norm never blocks the PE
            # queue while chunk 0's work is ready ----
            gact = persist.tile([P, c.FT, c.T], bf16, tag="gact",
                                name=f"gact{l}")
            xgds = []
            for b in range(c.B):
                emit_norm(xgs[b][:], 2 * l + 1, c.S, b * c.S, [(0, c.S)],
                          tag=f"no{l}{b}")
                for phase, wsrc3 in (("g", wg[l]), ("u", wu[l])):
                    gg = [(ot * P, P, c.S, b) for ot in range(c.FT)]
                    grhs = (lambda kt, rk, rl:
                            xn[:, kt, rk * c.S:(rk + 1) * c.S])
                    gps = kouter_pass(
                        c.KT, lambda k0, n, _w=wsrc3: _w[:, k0:k0 + n, :],
                        c.FPC, gg, grhs, name=f"{phase}{l}{b}")
                    for gi, (c0, cw, n, rk) in enumerate(gg):
                        ot = c0 // P
                        gsl = gact[:, ot, b * c.S:(b + 1) * c.S]
                        if phase == "g":
                            sgt = xpool.tile([P, c.S], bf16, tag="sgt",
                                             name="sgt", bufs=2)
                            nc.scalar.activation(sgt[:], gps[gi][:],
                                                 AF.Sigmoid)
                            nc.vector.tensor_mul(gsl, gps[gi][:], sgt[:])
                        else:
                            nc.vector.tensor_mul(gsl, gps[gi][:], gsl)
                # down partial for this chunk
                xpd = dram.tile([c.H, c.S], bf16, tag=f"xpd{l}_{b}",
                                name=f"xpd{l}_{b}")
                for m0 in range(0, c.MT, 8):
                    mtn = min(8, c.MT - m0)
                    dg = [((m0 + mi) * P, P, c.S, b) for mi in range(mtn)]
                    dps_ = kouter_pass(
                        c.FT,
                        lambda k0, n, _m0=m0, _mtn=mtn:
                            wd[l][:, k0:k0 + n, _m0 * P:(_m0 + _mtn) * P],
                        mtn * P,
                        [(mi * P, P, c.S, b) for mi in range(mtn)],
                        lambda kt, rk, rl, _b=b:
                            gact[:, kt, _b * c.S:(_b + 1) * c.S],
                        name=f"dp{l}{b}{m0}")
                    part_store(dps_, m0, xpd, c.S, tag=f"d{l}{b}{m0}")
                xgds.append(rs_add_ag(xpd, b, tag=f"d{l}_{b}"))
            if l == c.L - 2:
                xlraw = persist.tile([P, c.KT, c.B], bf16, tag="xlraw",
                                     name="xlraw")
            if l < c.L - 2:
                # next layer is full: norm both chunks here (its qkv pass
                # consumes both). Only the last transition interleaves with
                # the slim layer's k/v below.
                for b in range(c.B):
                    emit_norm(xgds[b][:], 2 * (l + 1), c.S, b * c.S,
                              [(0, c.S)], tag=f"nd{l}{b}")

        # ================= slim last layer =================
        l = c.L - 1

        # ---- down-transition norm chunk b + k/v pass chunk b interleaved --
        q_rot2 = persist.tile([c.HD, c.HPC, c.B], bf16, tag="qrot2",
                              name="qrot2")
        k_rot = persist.tile([c.HD, c.HPC, c.T], bf16, tag="krot",
                             name=f"krot{l}")
        v_sb = persist.tile([P, c.TP_, c.DR], bf16, tag="vsb",
                            name=f"vsb{l}")
        for b in range(c.B):
            emit_norm(xgds[b][:], 2 * l, c.S, b * c.S, [(0, c.S)],
                      tag=f"nd{b}", cap_dst=xlraw, cap_col=b)
            kg = [(h * c.HD, c.HD, c.S, b) for h in range(c.HPC)]
            krhs = lambda kt, rk, rl: xn[:, kt, rk * c.S:(rk + 1) * c.S]
            kps = kouter_pass(c.KT, lambda k0, n: wk[l][:, k0:k0 + n, :],
                              c.DR, kg, krhs, name=f"kp2{b}")
            for gi, (c0, cw, n, rk) in enumerate(kg):
                h = c0 // c.HD
                emit_rope(kps[gi], k_rot[:, h, rk * c.S:(rk + 1) * c.S],
                          cos_sb[:], sin_sb[:], c.S)
            vps = [psum.tile([P, c.DR], f32, tag="ps", name=f"vp2{b}{tt}")
                   for tt in range(c.SP)]
            G = max(1, min(8, 2048 // c.DR))
            for k0 in range(0, c.KT, G):
                g_n = min(G, c.KT - k0)
                wt = wpool.tile([P, G * c.DR], bf16, tag="wt", name="vw2")
                wt3 = wt[:].rearrange("p (g m) -> p g m", g=G)
                nc.sync.dma_start(out=wt3[:, 0:g_n, :],
                                  in_=wv[l][:, k0:k0 + g_n, :])
                for g in range(g_n):
                    kt = k0 + g
                    for tt in range(c.SP):
                        tg = b * c.SP + tt
                        nc.tensor.matmul(vps[tt][:],
                                         xn[:, kt, tg * P:(tg + 1) * P],
                                         wt3[:, g, :],
                                         start=(kt == 0), stop=(kt == c.KT - 1))
            for tt in range(c.SP):
                nc.scalar.copy(v_sb[:, b * c.SP + tt, :], vps[tt][:])

        # slim q: transposed pass -> qT [B, DR], then per-head transpose+rope
        xnl = persist.tile([P, c.KT, c.B], bf16, tag="xnl", name="xnl")
        nc.vector.tensor_copy(
            xnl[:],
            xn[:].rearrange("p kt (b s) -> p kt b s", s=c.S)[:, :, :, c.S - 1])
        qTp = psum.tile([c.B, c.DR], f32, tag="ps", name="qTp")
        G = max(1, min(8, 2048 // c.DR))
        for k0 in range(0, c.KT, G):
            g_n = min(G, c.KT - k0)
            wt = wpool.tile([P, G * c.DR], bf16, tag="wt", name="qw2")
            wt3 = wt[:].rearrange("p (g m) -> p g m", g=G)
            nc.sync.dma_start(out=wt3[:, 0:g_n, :],
                              in_=wq[l][:, k0:k0 + g_n, :])
            for g in range(g_n):
                kt = k0 + g
                nc.tensor.matmul(qTp[:], xnl[:, kt, :], wt3[:, g, :],
                                 start=(kt == 0), stop=(kt == c.KT - 1))
        qTr = spool.tile([c.B, c.DR], f32, tag="qTr", name="qTr", bufs=1)
        nc.vector.tensor_copy(qTr[:], qTp[:])
        for h in range(c.HPC):
            qhp = psum.tile([c.HD, c.B], f32, tag="ps", name=f"qhp{h}")
            nc.tensor.matmul(qhp[:], qTr[:, h * c.HD:(h + 1) * c.HD],
                             id2f_sb[:], is_transpose=True,
                             start=True, stop=True)
            emit_rope(qhp, q_rot2[:, h, :], cos2_sb[:], sin2_sb[:], c.B)

        # ---- slim attention -> ctx_lastT tiles [P, DRT, B] ----
        ctxL = persist.tile([P, c.DRT, c.B], bf16, tag="ctxL", name="ctxL")
        for b in range(c.B):
            for h in range(c.HPC):
                sps = psum.tile([P, c.SP], f32, tag="ps", name="sps2")
                for t in range(c.SP):
                    nc.tensor.matmul(
                        sps[:, t:t + 1],
                        k_rot[:, h, b * c.S + t * P:b * c.S + (t + 1) * P],
                        q_rot2[:, h, b:b + 1],
                        start=True, stop=True)
                pt = ppool.tile([P, c.SP], bf16, tag="pt", name="pt2")
                nc.scalar.activation(pt[:], sps[:], AF.Exp, scale=isqrt_hd)
                nc.vector.tensor_mul(pt[:], pt[:], am2_sb[:, b, :])
                dps = psum.tile([1, c.SP], f32, tag="ps", name="dps")
                nc.tensor.matmul(dps[:], ones_cbf[:], pt[:],
                                 start=True, stop=True)
                d1 = spool.tile([1, 1], f32, tag="d1", name="d1")
                nc.vector.tensor_reduce(d1[:], dps[:],
                                        mybir.AxisListType.X, OP.add)
                r1 = spool.tile([1, 1], f32, tag="r1", name="r1")
                nc.vector.reciprocal(r1[:], d1[:])
                cps = psum.tile([c.HD, 1], f32, tag="ps", name="cps2")
                for t in range(c.SP):
                    nc.tensor.matmul(
                        cps[:],
                        v_sb[:, b * c.SP + t, h * c.HD:(h + 1) * c.HD],
                        pt[:, t:t + 1],
                        start=(t == 0), stop=(t == c.SP - 1))
                bb = psum.tile([c.HD, 1], f32, tag="ps", name="bb2")
                nc.tensor.matmul(bb[:], ones_r32[:, 0:c.HD], r1[:],
                                 start=True, stop=True)
                bsb = spool.tile([c.HD, 1], f32, tag="bsb2", name="bsb2")
                nc.vector.tensor_copy(bsb[:], bb[:])
                csb = spool.tile([c.HD, 1], bf16, tag="csb2", name="csb2")
                nc.vector.tensor_mul(csb[:], cps[:], bsb[:])
                r0 = h * c.HD
                while r0 < (h + 1) * c.HD:
                    kt = r0 // P
                    pr0 = r0 % P
                    take = min((h + 1) * c.HD - r0, P - pr0)
                    nc.sync.dma_start(
                        out=ctxL[pr0:pr0 + take, kt, b:b + 1],
                        in_=csb[r0 - h * c.HD:r0 - h * c.HD + take, :])
                    r0 += take

        # ---- slim o partial (row-oriented, streamed weights) + AR1 ----
        olrow = spool.tile([c.B, c.H], f32, tag="olrow", name="olrow",
                           bufs=1)
        ops2 = [psum.tile([c.B, cw], f32, tag="ps", name=f"os{ci}")
                for ci, (c0, cw) in enumerate(HCH)]
        for kt in range(c.DRT):
            for h0 in range(0, len(HCH), 3):
                his = HCH[h0:h0 + 3]
                wcols = sum(cw for _, cw in his)
                wt = wpool.tile([P, wcols], bf16, tag="wt", name="ow2")
                nc.sync.dma_start(
                    out=wt[:],
                    in_=wo[l][:, kt, his[0][0]:his[0][0] + wcols])
                for ci, (c0, cw) in enumerate(his):
                    nc.tensor.matmul(ops2[h0 + ci][:], ctxL[:, kt, :],
                                     wt[:, c0 - his[0][0]:c0 - his[0][0] + cw],
                                     start=(kt == 0), stop=(kt == c.DRT - 1))
        # fold residual x_last/NC (transpose own raw tiles inline)
        for ci, (c0, cw) in enumerate(HCH):
            xls = spool.tile([c.B, cw], bf16, tag="sg2", name="xls", bufs=1)
            for j in range(cw // P):
                tp = psum.tile([c.B, P], bf16, tag="ps", name=f"xlt{ci}{j}")
                nc.tensor.matmul(tp[:], xlraw[:, c0 // P + j, :],
                                 id128b_sb[:], is_transpose=True,
                                 start=True, stop=True)
                nc.vector.tensor_copy(xls[:, j * P:(j + 1) * P], tp[:])
            nc.vector.scalar_tensor_tensor(
                olrow[:, c0:c0 + cw], xls[:], invnc_col[:],
                ops2[ci][:], OP.mult, OP.add)
        olb = dram.tile([c.B, c.H], f32, tag="olb", name="olb")
        nc.sync.dma_start(out=olb[:], in_=olrow[:])
        xlg2 = dram.tile([c.B, c.H], f32, addr_space=SHARED,
                         tag="xlg2", name="xlg2")
        nc.gpsimd.collective_compute(
            "AllReduce", OP.add, replica_groups=RG,
            ins=[olb[:]], outs=[xlg2[:]])
        x2row = spool.tile([c.B, c.H], f32, tag="x2row", name="x2row",
                           bufs=1)
        nc.sync.dma_start(out=x2row[:], in_=xlg2[:])

        def row_rmsnorm(src_row, dst_row, tag):
            """src [B, H] bf16 -> dst [B, H] bf16 (rmsnorm, no weight)."""
            sq = spool.tile([c.B, c.H], bf16, tag="rsq", name=f"rsq{tag}",
                            bufs=1)
            nc.vector.tensor_mul(sq[:], src_row, src_row)
            ssr = spool.tile([c.B, 1], f32, tag="ssr", name=f"ssr{tag}")
            nc.vector.tensor_reduce(ssr[:], sq[:], mybir.AxisListType.X,
                                    OP.add)
            lt = spool.tile([c.B, 1], f32, tag="lt2", name=f"lt2{tag}")
            nc.scalar.activation(lt[:], ssr[:], AF.Ln,
                                 bias=eps_col[0:c.B, :], scale=1.0 / c.H)
            rt = spool.tile([c.B, 1], f32, tag="rt2", name=f"rt2{tag}")
            nc.scalar.activation(rt[:], lt[:], AF.Exp, scale=-0.5)
            nc.vector.tensor_scalar(dst_row, src_row, rt[:], None, OP.mult)

        def row_to_tiles(src_row, nt, dst, tag, lnidx=None, gb=None):
            """[B, nt*P] bf16 row -> dst [P, nt, B] via PE transposes,
            optionally scaling by per-partition column weights."""
            for j in range(nt):
                tp = psum.tile([P, c.B], bf16, tag="ps", name=f"tt{tag}{j}")
                nc.tensor.matmul(tp[:], src_row[:, j * P:(j + 1) * P],
                                 id2b_sb[:], is_transpose=True,
                                 start=True, stop=True)
                if lnidx is not None:
                    nc.vector.tensor_scalar(dst[:, j, :], tp[:],
                                            lnw_sb[:, lnidx, j:j + 1], None,
                                            OP.mult)
                elif gb is not None:
                    nc.vector.tensor_scalar(dst[:, j, :], tp[:],
                                            gb[0][:, j:j + 1],
                                            gb[1][:, j:j + 1],
                                            OP.mult, OP.add)
                else:
                    nc.vector.tensor_copy(dst[:, j, :], tp[:])

        # ---- slim ln2 + gated MLP ----
        xn2row = spool.tile([c.B, c.H], bf16, tag="xn2row", name="xn2row",
                            bufs=1)
        row_rmsnorm(x2row[:], xn2row[:], tag="l2")
        xn2T = persist.tile([P, c.KT, c.B], bf16, tag="xnl", name="xn2T")
        row_to_tiles(xn2row[:], c.KT, xn2T, tag="x2", lnidx=2 * c.L - 1)

        garow = spool.tile([c.B, c.FPC], bf16, tag="garow", name="garow",
                           bufs=1)
        for phase, wsrc3 in (("g", wg[l]), ("u", wu[l])):
            gps2 = [psum.tile([c.B, cw], f32, tag="ps", name=f"g2{phase}{ci}")
                    for ci, (c0, cw) in enumerate(FCH)]
            G = max(1, min(8, 2048 // c.FPC))
            for k0 in range(0, c.KT, G):
                g_n = min(G, c.KT - k0)
                wt = wpool.tile([P, G * c.FPC], bf16, tag="wt",
                                name=f"g2w{phase}")
                wt3 = wt[:].rearrange("p (g m) -> p g m", g=G)
                nc.sync.dma_start(out=wt3[:, 0:g_n, :],
                                  in_=wsrc3[:, k0:k0 + g_n, :])
                for g in range(g_n):
                    kt = k0 + g
                    for ci, (c0, cw) in enumerate(FCH):
                        nc.tensor.matmul(gps2[ci][:], xn2T[:, kt, :],
                                         wt3[:, g, c0:c0 + cw],
                                         start=(kt == 0),
                                         stop=(kt == c.KT - 1))
            for ci, (c0, cw) in enumerate(FCH):
                if phase == "g":
                    sgt = spool.tile([c.B, cw], bf16, tag="sg2", name="sg2",
                                     bufs=1)
                    nc.scalar.activation(sgt[:], gps2[ci][:], AF.Sigmoid)
                    nc.vector.tensor_mul(garow[:, c0:c0 + cw], gps2[ci][:],
                                         sgt[:])
                else:
                    nc.vector.tensor_mul(garow[:, c0:c0 + cw],
                                         gps2[ci][:], garow[:, c0:c0 + cw])
        gactT = persist.tile([P, c.FT, c.B], bf16, tag="gactT", name="gactT")
        row_to_tiles(garow[:], c.FT, gactT, tag="ga")

        # ---- slim down partial + AR2 ----
        dlrow = spool.tile([c.B, c.H], f32, tag="olrow", name="dlrow",
                           bufs=1)
        dps2 = [psum.tile([c.B, cw], f32, tag="ps", name=f"ds{ci}")
                for ci, (c0, cw) in enumerate(HCH)]
        for kt in range(c.FT):
            for h0 in range(0, len(HCH), 3):
                his = HCH[h0:h0 + 3]
                wcols = sum(cw for _, cw in his)
                wt = wpool.tile([P, wcols], bf16, tag="wt", name="d2w")
                nc.sync.dma_start(
                    out=wt[:],
                    in_=wd[l][:, kt, his[0][0]:his[0][0] + wcols])
                for ci, (c0, cw) in enumerate(his):
                    nc.tensor.matmul(dps2[h0 + ci][:], gactT[:, kt, :],
                                     wt[:, c0 - his[0][0]:c0 - his[0][0] + cw],
                                     start=(kt == 0), stop=(kt == c.FT - 1))
        for ci, (c0, cw) in enumerate(HCH):
            nc.vector.scalar_tensor_tensor(
                dlrow[:, c0:c0 + cw], x2row[:, c0:c0 + cw], invnc_col[:],
                dps2[ci][:], OP.mult, OP.add)
        dlb = dram.tile([c.B, c.H], f32, tag="dlb", name="dlb")
        nc.sync.dma_start(out=dlb[:], in_=dlrow[:])
        xfing = dram.tile([c.B, c.H], f32, addr_space=SHARED,
                          tag="xfing", name="xfing")
        nc.gpsimd.collective_compute(
            "AllReduce", OP.add, replica_groups=RG,
            ins=[dlb[:]], outs=[xfing[:]])
        xfrow = spool.tile([c.B, c.H], f32, tag="x2row", name="xfrow",
                           bufs=1)
        nc.sync.dma_start(out=xfrow[:], in_=xfing[:])

        # ================= final norm + cls head =================
        xfn = spool.tile([c.B, c.H], bf16, tag="xn2row", name="xfn",
                         bufs=1)
        row_rmsnorm(xfrow[:], xfn[:], tag="fin")
        xnf = persist.tile([P, c.KT, c.B], bf16, tag="xnl", name="xnf")
        row_to_tiles(xfn[:], c.KT, xnf, tag="xf", lnidx=2 * c.L)

        hps = [psum.tile([c.B, cw], f32, tag="ps", name=f"hp{ci}")
               for ci, (c0, cw) in enumerate(CCH)]
        G = max(1, min(8, 2048 // c.CLS))
        for k0 in range(0, c.KT, G):
            g_n = min(G, c.KT - k0)
            wt = wpool.tile([P, G * c.CLS], bf16, tag="wt", name="w1w")
            wt3 = wt[:].rearrange("p (g m) -> p g m", g=G)
            nc.sync.dma_start(out=wt3[:, 0:g_n, :],
                              in_=w1t[:, k0:k0 + g_n, :])
            for g in range(g_n):
                kt = k0 + g
                for ci, (c0, cw) in enumerate(CCH):
                    nc.tensor.matmul(hps[ci][:], xnf[:, kt, :],
                                     wt3[:, g, c0:c0 + cw],
                                     start=(kt == 0), stop=(kt == c.KT - 1))
        hrow = spool.tile([c.B, c.CLS], f32, tag="hrow", name="hrow",
                          bufs=1)
        for ci, (c0, cw) in enumerate(CCH):
            t0_ = spool.tile([c.B, cw], f32, tag="hc", name="hc", bufs=1)
            nc.vector.tensor_add(t0_[:], hps[ci][:], b1row_sb[:, c0:c0 + cw])
            nc.scalar.activation(hrow[:, c0:c0 + cw], t0_[:], AF.Relu)
        mu = spool.tile([c.B, 1], f32, tag="mu", name="mu")
        nc.vector.tensor_reduce(mu[:], hrow[:], mybir.AxisListType.X, OP.add)
        nc.vector.tensor_scalar_mul(mu[:], mu[:], 1.0 / c.CLS)
        hsq = spool.tile([c.B, c.CLS], f32, tag="hsq", name="hsq", bufs=1)
        nc.vector.tensor_mul(hsq[:], hrow[:], hrow[:])
        s2 = spool.tile([c.B, 1], f32, tag="s2", name="s2")
        nc.vector.tensor_reduce(s2[:], hsq[:], mybir.AxisListType.X, OP.add)
        nc.vector.tensor_scalar_mul(s2[:], s2[:], 1.0 / c.CLS)
        msq = spool.tile([c.B, 1], f32, tag="msq", name="msq")
        nc.vector.tensor_mul(msq[:], mu[:], mu[:])
        var = spool.tile([c.B, 1], f32, tag="var", name="var")
        nc.vector.tensor_sub(var[:], s2[:], msq[:])
        lv = spool.tile([c.B, 1], f32, tag="lv", name="lv")
        nc.scalar.activation(lv[:], var[:], AF.Ln, bias=eps_col[0:c.B, :])
        rstd = spool.tile([c.B, 1], f32, tag="rstd", name="rstd")
        nc.scalar.activation(rstd[:], lv[:], AF.Exp, scale=-0.5)
        t1 = spool.tile([c.B, c.CLS], bf16, tag="t1", name="t1", bufs=1)
        nc.vector.tensor_scalar(t1[:], hrow[:], mu[:], rstd[:],
                                OP.subtract, OP.mult)
        hnT = persist.tile([P, c.CT, c.B], bf16, tag="hnT", name="hnT")
        row_to_tiles(t1[:], c.CT, hnT, tag="hn", gb=(gcol_sb, bcol_sb))

        lg = psum.tile([c.B, c.NCLS], f32, tag="ps", name="lg")
        for ot in range(c.CT):
            nc.tensor.matmul(lg[:], hnT[:, ot, :], w2t_sb[:, ot, :],
                             start=(ot == 0), stop=(ot == c.CT - 1))
        lg_sb = spool.tile([c.B, c.NCLS], f32, tag="lg_sb", name="lg_sb")
        nc.vector.tensor_add(lg_sb[:], lg[:], b2row_sb[:])
        nc.sync.dma_start(out=out_d, in_=lg_sb[:])

    nc.compile()
    return nc


# ----------------------------------------------------------------------------
# entry point
# ----------------------------------------------------------------------------

_CACHE = {}


def _get_nc(cfg):
    if cfg not in _CACHE:
        _CACHE[cfg] = build_nc(cfg)
    return _CACHE[cfg]


def run(cfg, inputs, trace=False, **kw):
    from concourse.bass_utils import run_bass_kernel_spmd
    in_maps = host_prep(cfg, inputs)
    nc = _get_nc(cfg)
    res = run_bass_kernel_spmd(nc, in_maps, core_ids=list(range(cfg.NC)),
                               trace=trace, **kw)
    out = np.asarray(res.results[0]["logits_out"])  # [B, NCLS]
    return np.ascontiguousarray(out.astype(np.float32)), res


def kernel(**inputs):
    inputs = {k: np.asarray(v) for k, v in inputs.items()}
    out, _ = run(FULL_CFG, inputs)
    return out
